# revision 37
# baseline (speedup 1.0000x reference)
import os
import sys

sys.path.insert(0, '/opt/trn_rl_repo')
import numpy as np
from concourse import bass, bacc, mybir
import concourse.tile as tile
from concourse.bass_utils import run_bass_kernel_spmd
from concourse.masks import make_identity

f32 = mybir.dt.float32
f32r = mybir.dt.float32r
i32 = mybir.dt.int32
AF = mybir.ActivationFunctionType
ALU = mybir.AluOpType

EPS = 1e-5
NCORES = 8
N = 50000
E = 500000
NPC = 6250            # nodes per core
NT = 13               # node tiles of 512 per core
NPAD = NT * 512       # 6656
NWIN_E = 49           # edge scatter windows of 128 rows (covers 6250)
NWIN_B = NT * 4       # 52 windows read back in phase B (49..51 are zero pad)

TRACE = bool(os.environ.get("K_TRACE"))
REDUCE = bool(os.environ.get("K_REDUCE"))  # tiny program for compile smoke only
DBG = bool(os.environ.get("K_DBG"))
last_exec_ns = []
dbg = {}

I3 = np.eye(3, dtype=np.float32)
R1 = np.zeros((3, 3), np.float32)
R2 = np.zeros((3, 3), np.float32)
for _c in range(3):
    R1[(_c + 1) % 3, _c] = 1
    R2[(_c + 2) % 3, _c] = 1


def _f32(a):
    return np.ascontiguousarray(np.asarray(a), dtype=np.float32)


def prep_simple(p):
    wh = _f32(p['wh']); ws = _f32(p['ws_w']); wv = _f32(p['wv']); wsv = _f32(p['wsv_w'])
    h = wh.shape[1]
    si = ws.shape[0] - h
    return dict(
        whk=_f32(np.kron(wh, I3)),            # [3vi, 3h]
        wsa=_f32(ws[:si]),                    # [si, 128]
        wsb=_f32(ws[si:]),                    # [h, 128]
        wvk=_f32(np.kron(wv, I3)),            # [3h, 3vo]
        wsvr=_f32(np.repeat(wsv, 3, 1)),      # [128, 3vo]
    )


def prep_fused(p):
    wh = _f32(p['wh']); ws = _f32(p['ws_w']); wv = _f32(p['wv']); wsv = _f32(p['wsv_w'])
    return dict(
        waf0=_f32(0.5 * np.kron(wh[:16], I3)),  # [48,96]
        waf1=_f32(np.kron(wh[16:], R1)),        # [48,96]
        bfold=_f32(-np.kron(wh[16:], R2)),      # [48, 96]
        ws_s=_f32(ws[:128]),
        ws_u=_f32(np.repeat(ws[128:144], 3, 0)),   # [48, 128]
        ws_vn=_f32(ws[144:176]),
        wvk=_f32(np.kron(wv, I3)),            # [96, 48]
        wsvr=_f32(np.repeat(wsv, 3, 1)),
    )


ONES3_16 = _f32(np.kron(np.eye(16, dtype=np.float32), np.ones((3, 1), np.float32)))
ONES3_32 = _f32(np.kron(np.eye(32, dtype=np.float32), np.ones((3, 1), np.float32)))
ROTM1 = _f32(np.kron(np.eye(16, dtype=np.float32), R1))   # [48,48]
ROTM2 = _f32(np.kron(np.eye(16, dtype=np.float32), R2))   # [48,48]
VBM = _f32(np.kron(np.ones((16, 1), np.float32) / 16.0, I3))   # [48,3]
VEXP = _f32(np.kron(np.ones((1, 16), np.float32), I3))          # [3,48]
IOTA = _f32(np.broadcast_to(np.arange(128, dtype=np.float32), (128, 128)))


class Prog:
    def __init__(self):
        self.nc = bacc.Bacc(None, target_bir_lowering=False)
        self.common = {}

    def cin(self, name, arr, dtype):
        arr = np.ascontiguousarray(arr)
        t = self.nc.dram_tensor(name, list(arr.shape), dtype, kind="ExternalInput")
        self.common[name] = arr
        return t


def load_wset(nc, wp, pd, tag, W, dtype=f32r):
    out = {}
    for k, arr in W.items():
        d = pd.cin(f"{tag}_{k}", arr, dtype)
        s = wp.tile(list(arr.shape), dtype, name=f"{tag}_{k}")
        nc.sync.dma_start(out=s[:], in_=d[:])
        out[k] = s
    return out


def alloc_PS(psp, names):
    return {n: psp.tile([128, 512], f32, name=f"ps_{n}") for n in names}


def emit_gvp_simple(nc, dp, PS, W, ones3, s_t, v_t, relu, s_dtype, v_dtype):
    nc.tensor.matmul(out=PS['vh'][:48, :], lhsT=W['whk'][:], rhs=v_t, start=True, stop=True)
    vh_s = dp.tile([48, 512], f32r, name="vh_s")
    nc.scalar.copy(vh_s[:], PS['vh'][:48, :])
    sq_s = dp.tile([48, 512], f32r, name="sq_s")
    nc.scalar.square(sq_s[:], PS['vh'][:48, :])
    nc.tensor.matmul(out=PS['vnsq'][:16, :], lhsT=ones3[:], rhs=sq_s[:], start=True, stop=True)
    vnp_s = dp.tile([16, 512], f32, name="vnp_s")
    nc.vector.tensor_scalar_max(out=vnp_s[:], in0=PS['vnsq'][:16, :], scalar1=EPS)
    vn_s = dp.tile([16, 512], f32r, name="vn_s")
    nc.scalar.sqrt(vn_s[:], vnp_s[:])
    nc.tensor.matmul(out=PS['s'][:, :], lhsT=W['wsa'][:], rhs=s_t, start=True, stop=False)
    nc.tensor.matmul(out=PS['s'][:, :], lhsT=W['wsb'][:], rhs=vn_s[:], start=False, stop=True)
    s_out = dp.tile([128, 512], s_dtype, name="so_s")
    nc.scalar.activation(s_out[:], PS['s'][:, :], AF.Relu if relu else AF.Identity)
    ssig = dp.tile([128, 512], f32r, name="ssig_s")
    nc.scalar.activation(ssig[:], PS['s'][:, :], AF.Sigmoid)
    nc.tensor.matmul(out=PS['vo'][:48, :], lhsT=W['wvk'][:], rhs=vh_s[:], start=True, stop=True)
    nc.tensor.matmul(out=PS['g'][:48, :], lhsT=W['wsvr'][:], rhs=ssig[:], start=True, stop=True)
    gs_s = dp.tile([48, 512], f32, name="gs_s")
    nc.scalar.activation(gs_s[:], PS['g'][:48, :], AF.Sigmoid)
    v_out = dp.tile([48, 512], v_dtype, name="vo_s")
    nc.vector.tensor_tensor(out=v_out[:], in0=PS['vo'][:48, :], in1=gs_s[:], op=ALU.mult)
    return s_out, v_out


def emit_gvp_fused(nc, dp, PS, W, ones3, rotm1, rotm2, ls, lv, rs, rv, relu,
                   s_dtype, v_dtype):
    fss = dp.tile([128, 512], f32r, name="fss_f")
    nc.vector.tensor_tensor(out=fss[:], in0=ls, in1=rs, op=ALU.mult)
    u_f = dp.tile([48, 512], f32r, name="u_f")
    nc.vector.tensor_tensor(out=u_f[:], in0=lv, in1=rv, op=ALU.mult)
    nc.tensor.matmul(out=PS['rot'][:48, :], lhsT=rotm1[:], rhs=rv, start=True, stop=True)
    nc.tensor.matmul(out=PS['vnsq'][:48, :], lhsT=rotm2[:], rhs=rv, start=True, stop=True)
    wa0_f = dp.tile([48, 512], f32r, name="wa0_f")
    nc.vector.tensor_tensor(out=wa0_f[:], in0=lv, in1=rv, op=ALU.add)
    wa1_f = dp.tile([48, 512], f32r, name="wa1_f")
    nc.vector.tensor_tensor(out=wa1_f[:], in0=lv, in1=PS['rot'][:48, :], op=ALU.mult)
    bt_f = dp.tile([48, 512], f32r, name="bt_f")
    nc.vector.tensor_tensor(out=bt_f[:], in0=lv, in1=PS['vnsq'][:48, :], op=ALU.mult)
    nc.tensor.matmul(out=PS['vh'][:96, :], lhsT=W['waf0'][:], rhs=wa0_f[:], start=True, stop=False)
    nc.tensor.matmul(out=PS['vh'][:96, :], lhsT=W['waf1'][:], rhs=wa1_f[:], start=False, stop=False)
    nc.tensor.matmul(out=PS['vh'][:96, :], lhsT=W['bfold'][:], rhs=bt_f[:], start=False, stop=True)
    vh_f = dp.tile([96, 512], f32r, name="vh_f")
    nc.scalar.copy(vh_f[:], PS['vh'][:96, :])
    sq_f = dp.tile([96, 512], f32r, name="sq_f")
    nc.scalar.square(sq_f[:], PS['vh'][:96, :])
    nc.tensor.matmul(out=PS['vnsq'][:32, :], lhsT=ones3[:], rhs=sq_f[:], start=True, stop=True)
    vnp_f = dp.tile([32, 512], f32, name="vnp_f")
    nc.vector.tensor_scalar_max(out=vnp_f[:], in0=PS['vnsq'][:32, :], scalar1=EPS)
    vn_f = dp.tile([32, 512], f32r, name="vn_f")
    nc.scalar.sqrt(vn_f[:], vnp_f[:])
    nc.tensor.matmul(out=PS['s'][:, :], lhsT=W['ws_s'][:], rhs=fss[:], start=True, stop=False)
    nc.tensor.matmul(out=PS['s'][:, :], lhsT=W['ws_u'][:], rhs=u_f[:], start=False, stop=False)
    nc.tensor.matmul(out=PS['s'][:, :], lhsT=W['ws_vn'][:], rhs=vn_f[:], start=False, stop=True)
    s_out = dp.tile([128, 512], s_dtype, name="so_f")
    nc.scalar.activation(s_out[:], PS['s'][:, :], AF.Relu if relu else AF.Identity)
    ssig = dp.tile([128, 512], f32r, name="ssig_f")
    nc.scalar.activation(ssig[:], PS['s'][:, :], AF.Sigmoid)
    nc.tensor.matmul(out=PS['vo'][:48, :], lhsT=W['wvk'][:], rhs=vh_f[:], start=True, stop=True)
    nc.tensor.matmul(out=PS['g'][:48, :], lhsT=W['wsvr'][:], rhs=ssig[:], start=True, stop=True)
    gs_f = dp.tile([48, 512], f32, name="gs_f")
    nc.scalar.activation(gs_f[:], PS['g'][:48, :], AF.Sigmoid)
    v_out = dp.tile([48, 512], v_dtype, name="vo_f")
    nc.vector.tensor_tensor(out=v_out[:], in0=PS['vo'][:48, :], in1=gs_f[:], op=ALU.mult)
    return s_out, v_out


def build_phase_a(Wn0, Wn1):
    pd = Prog()
    nc = pd.nc
    xs_d = nc.dram_tensor("xs", [128, NPAD], f32r, kind="ExternalInput")
    xv_d = nc.dram_tensor("xv", [48, NPAD], f32r, kind="ExternalInput")
    tbl_d = nc.dram_tensor("tbl", [NPAD, 176], f32, kind="ExternalOutput")
    nt = 1 if REDUCE else NT

    with tile.TileContext(nc) as tc:
        with tc.tile_pool(name="wp", bufs=1) as wp, \
             tc.tile_pool(name="dp", bufs=2) as dp, \
             tc.tile_pool(name="psp", bufs=1, space="PSUM") as psp:
            PS = alloc_PS(psp, ["vh", "vnsq", "s", "vo", "g", "tp"])
            W0 = load_wset(nc, wp, pd, "n0", Wn0)
            W1 = load_wset(nc, wp, pd, "n1", Wn1)
            o3_d = pd.cin("o3_16", ONES3_16, f32r)
            o3 = wp.tile([48, 16], f32r, name="o3_16")
            nc.sync.dma_start(out=o3[:], in_=o3_d[:])
            ident = wp.tile([128, 128], f32, name="ident")
            make_identity(nc, ident[:])

            for t in range(nt):
                sl = slice(t * 512, (t + 1) * 512)
                xs_t = dp.tile([128, 512], f32r, name="xs_t")
                nc.sync.dma_start(out=xs_t[:], in_=xs_d[:, sl])
                xv_t = dp.tile([48, 512], f32r, name="xv_t")
                nc.sync.dma_start(out=xv_t[:], in_=xv_d[:, sl])
                hs0, hv0 = emit_gvp_simple(nc, dp, PS, W0, o3, xs_t[:], xv_t[:], True, f32r, f32r)
                hs1, hv1 = emit_gvp_simple(nc, dp, PS, W1, o3, hs0[:], hv0[:], False, f32, f32)
                for g in range(4):
                    gsl = slice(g * 128, (g + 1) * 128)
                    nc.tensor.transpose(out=PS['tp'][:, 0:128], in_=hs1[:, gsl], identity=ident[:])
                    nc.tensor.transpose(out=PS['tp'][:, 128:176], in_=hv1[:, gsl], identity=ident[:48, :48])
                    slab = dp.tile([128, 176], f32, name="slab", bufs=4)
                    nc.scalar.copy(slab[:], PS['tp'][:, 0:176])
                    r0 = t * 512 + g * 128
                    nc.sync.dma_start(out=tbl_d[r0:r0 + 128, :], in_=slab[:])
    nc.compile()
    return pd


def build_main(C, We0, We1, Wm, Wc, Wo0, Wo1, ln_w, ln_b):
    G = NWIN_E * C
    Ec = G * 128
    TPW = C // 4
    pd = Prog()
    nc = pd.nc
    tbl_d = nc.dram_tensor("tbl", [N, 176], f32, kind="ExternalInput")
    eas_d = nc.dram_tensor("eas", [64, Ec], f32r, kind="ExternalInput")
    eav_d = nc.dram_tensor("eav", [48, Ec], f32r, kind="ExternalInput")
    col_d = nc.dram_tensor("colv", [128, G], i32, kind="ExternalInput")
    rr_d = nc.dram_tensor("rrv", [128, G], f32, kind="ExternalInput")
    xs_d = nc.dram_tensor("xs", [128, NPAD], f32r, kind="ExternalInput")
    xv_d = nc.dram_tensor("xv", [48, NPAD], f32r, kind="ExternalInput")
    out_d = nc.dram_tensor("out", [NPAD, 176], f32, kind="ExternalOutput")
    dbgacc_d = (nc.dram_tensor("dbgacc", [128, NWIN_B * 176], f32,
                               kind="ExternalOutput") if DBG else None)
    if DBG:
        dbg_d = {nm: nc.dram_tensor(f"dbg_{nm}", [p, 512], dt, kind="ExternalOutput")
                 for nm, p, dt in [("hes", 128, f32r), ("hev", 48, f32r),
                                   ("rs", 128, f32r), ("rv", 48, f32r),
                                   ("ms", 128, f32), ("mv", 48, f32)]}

    nwin_e = 1 if REDUCE else NWIN_E
    nt_b = 1 if REDUCE else NT

    with tile.TileContext(nc) as tc:
        with tc.tile_pool(name="wp", bufs=1) as wp, \
             tc.tile_pool(name="ap", bufs=1) as ap, \
             tc.tile_pool(name="dp", bufs=2) as dp, \
             tc.tile_pool(name="bp", bufs=1) as bp, \
             tc.tile_pool(name="psp", bufs=1, space="PSUM") as psp:
            PS = alloc_PS(psp, ["vh", "vnsq", "s", "vo", "g", "rot", "tp", "win"])
            WE0 = load_wset(nc, wp, pd, "e0", We0)
            WE1 = load_wset(nc, wp, pd, "e1", We1)
            WM = load_wset(nc, wp, pd, "m", Wm)
            WC = load_wset(nc, wp, pd, "c", Wc)
            WO0 = load_wset(nc, wp, pd, "o0", Wo0)
            WO1 = load_wset(nc, wp, pd, "o1", Wo1)

            def cin_tile(name, arr, dtype):
                d = pd.cin(name, arr, dtype)
                s = wp.tile(list(arr.shape), dtype, name=name)
                nc.sync.dma_start(out=s[:], in_=d[:])
                return s

            o3_16 = cin_tile("o3_16", ONES3_16, f32r)
            o3_32 = cin_tile("o3_32", ONES3_32, f32r)
            rotm1 = cin_tile("rotm1", ROTM1, f32r)
            rotm2 = cin_tile("rotm2", ROTM2, f32r)
            iota_t = cin_tile("iota", IOTA, f32)
            vbm_t = cin_tile("vbm", VBM, f32)
            vexp_t = cin_tile("vexp", VEXP, f32)
            onesd_t = cin_tile("onesd", np.full((128, 1), 1.0 / 128.0, np.float32), f32)
            ones1_t = cin_tile("ones1", np.ones((1, 128), np.float32), f32)
            ones1x_t = cin_tile("ones1x", np.ones((33, 128), np.float32), f32)
            _vexpx = np.zeros((67, 48), np.float32)
            _vexpx[64:67] = VEXP
            vexpx_t = cin_tile("vexpx", _vexpx, f32)
            lnw_t = cin_tile("lnw", ln_w.reshape(128, 1), f32)
            lnb_t = cin_tile("lnb", ln_b.reshape(128, 1), f32)
            epsb_t = cin_tile("epsb", np.full((128, 1), EPS, np.float32), f32)
            ident = wp.tile([128, 128], f32, name="ident")
            make_identity(nc, ident[:])

            col_all = ap.tile([128, G], i32, name="col_all")
            nc.sync.dma_start(out=col_all[:], in_=col_d[:])
            rr_all = ap.tile([128, G], f32, name="rr_all")
            nc.sync.dma_start(out=rr_all[:], in_=rr_d[:])
            acc_sb = ap.tile([128, NWIN_B * 176], f32, name="acc_sb")
            nc.vector.memset(acc_sb[:], 0.0)

            # ---- edge phase ----
            for w in range(nwin_e):
                for ti in range(TPW):
                    t = w * TPW + ti
                    sl = slice(t * 512, (t + 1) * 512)
                    eas_t = dp.tile([64, 512], f32r, name="eas_t")
                    nc.sync.dma_start(out=eas_t[:], in_=eas_d[:, sl])
                    eav_t = dp.tile([48, 512], f32r, name="eav_t")
                    nc.sync.dma_start(out=eav_t[:], in_=eav_d[:, sl])
                    hs0, hv0 = emit_gvp_simple(nc, dp, PS, WE0, o3_16,
                                               eas_t[:], eav_t[:], True, f32r, f32r)
                    hes, hev = emit_gvp_simple(nc, dp, PS, WE1, o3_16,
                                               hs0[:], hv0[:], False, f32r, f32r)
                    rs_t = dp.tile([128, 512], f32r, name="rs_t")
                    rv_t = dp.tile([48, 512], f32r, name="rv_t")
                    for g in range(4):
                        gidx = t * 4 + g
                        gsl = slice(g * 128, (g + 1) * 128)
                        ghn = dp.tile([128, 176], f32, name="ghn", bufs=4)
                        nc.gpsimd.indirect_dma_start(
                            out=ghn[:], out_offset=None, in_=tbl_d[:],
                            in_offset=bass.IndirectOffsetOnAxis(
                                ap=col_all[:, gidx:gidx + 1], axis=0))
                        nc.tensor.transpose(out=PS['tp'][:, 0:128], in_=ghn[:, 0:128],
                                            identity=ident[:])
                        nc.tensor.transpose(out=PS['tp'][:48, 128:256], in_=ghn[:, 128:176],
                                            identity=ident[:])
                        nc.scalar.copy(rs_t[:, gsl], PS['tp'][:, 0:128])
                        nc.scalar.copy(rv_t[:, gsl], PS['tp'][:48, 128:256])
                    ms, mv = emit_gvp_fused(nc, dp, PS, WM, o3_32, rotm1, rotm2,
                                            hes[:], hev[:], rs_t[:], rv_t[:], False, f32, f32)
                    if DBG and t == 0:
                        for nm, src in [("hes", hes), ("hev", hev), ("rs", rs_t),
                                        ("rv", rv_t), ("ms", ms), ("mv", mv)]:
                            nc.sync.dma_start(out=dbg_d[nm][:], in_=src[:])
                    for g in range(4):
                        gidx = t * 4 + g
                        gsl = slice(g * 128, (g + 1) * 128)
                        nc.tensor.transpose(out=PS['tp'][:, 256:384], in_=ms[:, gsl],
                                            identity=ident[:])
                        nc.tensor.transpose(out=PS['tp'][:, 384:432], in_=mv[:, gsl],
                                            identity=ident[:48, :48])
                        mT = dp.tile([128, 176], f32, name="mT", bufs=2)
                        nc.scalar.copy(mT[:], PS['tp'][:, 256:432])
                        sel = dp.tile([128, 128], f32, name="sel", bufs=2)
                        nc.vector.tensor_tensor(
                            out=sel[:], in0=rr_all[:, gidx:gidx + 1].to_broadcast([128, 128]),
                            in1=iota_t[:], op=ALU.is_equal)
                        nc.tensor.matmul(out=PS['win'][:, 0:176], lhsT=sel[:], rhs=mT[:],
                                         start=(ti == 0 and g == 0),
                                         stop=(ti == TPW - 1 and g == 3))
                nc.scalar.copy(acc_sb[:, w * 176:(w + 1) * 176], PS['win'][:, 0:176])

            if DBG:
                nc.sync.dma_start(out=dbgacc_d[:], in_=acc_sb[:])

            # ---- node phase B ----
            for t in range(nt_b):
                sl = slice(t * 512, (t + 1) * 512)
                xs_t = dp.tile([128, 512], f32r, name="xs_t")
                nc.sync.dma_start(out=xs_t[:], in_=xs_d[:, sl])
                xv_t = dp.tile([48, 512], f32r, name="xv_t")
                nc.sync.dma_start(out=xv_t[:], in_=xv_d[:, sl])
                asv = dp.tile([128, 512], f32r, name="asv")
                avv = dp.tile([48, 512], f32r, name="avv")
                for g in range(4):
                    w = t * 4 + g
                    gsl = slice(g * 128, (g + 1) * 128)
                    nc.tensor.transpose(out=PS['tp'][:, 0:128],
                                        in_=acc_sb[:, w * 176:w * 176 + 128],
                                        identity=ident[:])
                    nc.tensor.transpose(out=PS['tp'][:48, 128:256],
                                        in_=acc_sb[:, w * 176 + 128:w * 176 + 176],
                                        identity=ident[:])
                    nc.scalar.copy(asv[:, gsl], PS['tp'][:, 0:128])
                    nc.scalar.copy(avv[:, gsl], PS['tp'][:48, 128:256])
                cs, cv = emit_gvp_simple(nc, dp, PS, WC, o3_16,
                                         xs_t[:], xv_t[:], False, f32r, f32r)
                os0, ov0 = emit_gvp_fused(nc, dp, PS, WO0, o3_32, rotm1, rotm2,
                                          cs[:], cv[:], asv[:], avv[:], True, f32r, f32r)
                os1, ov1 = emit_gvp_simple(nc, dp, PS, WO1, o3_16,
                                           os0[:], ov0[:], False, f32, f32)
                osr = bp.tile([128, 512], f32, name="osr")
                nc.vector.tensor_tensor(out=osr[:], in0=os1[:], in1=xs_t[:], op=ALU.add)
                ovr = bp.tile([48, 512], f32, name="ovr")
                nc.vector.tensor_tensor(out=ovr[:], in0=ov1[:], in1=xv_t[:], op=ALU.add)
                # scalar LN
                sqb = bp.tile([128, 512], f32, name="sqb")
                nc.scalar.square(sqb[:], osr[:])
                nc.tensor.matmul(out=PS['vo'][:1, :], lhsT=onesd_t[:], rhs=osr[:],
                                 start=True, stop=True)
                nc.tensor.matmul(out=PS['g'][:1, :], lhsT=onesd_t[:], rhs=sqb[:],
                                 start=True, stop=True)
                statA = bp.tile([128, 512], f32, name="statA")
                statB = bp.tile([128, 512], f32, name="statB")
                statC = bp.tile([128, 512], f32, name="statC")
                nc.scalar.copy(statA[0:1, :], PS['vo'][:1, :])
                nc.vector.tensor_tensor(out=statB[32:33, :], in0=statA[0:1, :],
                                        in1=statA[0:1, :], op=ALU.mult)
                nc.vector.tensor_tensor(out=statB[64:65, :], in0=PS['g'][:1, :],
                                        in1=statB[32:33, :], op=ALU.subtract)
                nc.scalar.activation(statB[96:97, :], statB[64:65, :], AF.Sqrt,
                                     bias=epsb_t[96:97, :])
                nc.vector.reciprocal(out=statA[32:33, :], in_=statB[96:97, :])
                nc.tensor.matmul(out=PS['s'][:, :], lhsT=ones1_t[:], rhs=statA[0:1, :],
                                 start=True, stop=True)
                nc.tensor.matmul(out=PS['vh'][:, :], lhsT=ones1x_t[32:33, :],
                                 rhs=statA[32:33, :], start=True, stop=True)
                t1 = bp.tile([128, 512], f32, name="t1")
                nc.vector.tensor_tensor(out=t1[:], in0=osr[:], in1=PS['s'][:, :],
                                        op=ALU.subtract)
                t2 = bp.tile([128, 512], f32, name="t2")
                nc.vector.tensor_tensor(out=t2[:], in0=t1[:], in1=PS['vh'][:, :],
                                        op=ALU.mult)
                osf = bp.tile([128, 512], f32, name="osf")
                nc.scalar.activation(osf[:], t2[:], AF.Identity, bias=lnb_t[:],
                                     scale=lnw_t[:])
                # vector LN
                sqv = bp.tile([48, 512], f32, name="sqv")
                nc.scalar.square(sqv[:], ovr[:])
                nc.tensor.matmul(out=PS['vo'][:3, :], lhsT=vbm_t[:], rhs=ovr[:],
                                 start=True, stop=True)
                nc.tensor.matmul(out=PS['g'][:3, :], lhsT=vbm_t[:], rhs=sqv[:],
                                 start=True, stop=True)
                nc.scalar.copy(statA[64:67, :], PS['vo'][:3, :])
                nc.vector.tensor_tensor(out=statC[0:3, :], in0=statA[64:67, :],
                                        in1=statA[64:67, :], op=ALU.mult)
                nc.vector.tensor_tensor(out=statC[32:35, :], in0=PS['g'][:3, :],
                                        in1=statC[0:3, :], op=ALU.subtract)
                nc.scalar.activation(statC[64:67, :], statC[32:35, :], AF.Sqrt,
                                     bias=epsb_t[64:67, :])
                nc.vector.reciprocal(out=statB[0:3, :], in_=statC[64:67, :])
                nc.tensor.matmul(out=PS['vo'][:48, :], lhsT=vexpx_t[64:67, :],
                                 rhs=statA[64:67, :], start=True, stop=True)
                nc.tensor.matmul(out=PS['g'][:48, :], lhsT=vexp_t[:], rhs=statB[0:3, :],
                                 start=True, stop=True)
                tv1 = bp.tile([48, 512], f32, name="tv1")
                nc.vector.tensor_tensor(out=tv1[:], in0=ovr[:], in1=PS['vo'][:48, :],
                                        op=ALU.subtract)
                ovf = bp.tile([48, 512], f32, name="ovf")
                nc.vector.tensor_tensor(out=ovf[:], in0=tv1[:], in1=PS['g'][:48, :],
                                        op=ALU.mult)
                for g in range(4):
                    gsl = slice(g * 128, (g + 1) * 128)
                    nc.tensor.transpose(out=PS['tp'][:, 256:384], in_=osf[:, gsl],
                                        identity=ident[:])
                    nc.tensor.transpose(out=PS['tp'][:, 384:432], in_=ovf[:, gsl],
                                        identity=ident[:48, :48])
                    onm = bp.tile([128, 176], f32, name="onm", bufs=4)
                    nc.scalar.copy(onm[:], PS['tp'][:, 256:432])
                    r0 = t * 512 + g * 128
                    nc.sync.dma_start(out=out_d[r0:r0 + 128, :], in_=onm[:])
    nc.compile()
    return pd


def _timed_pjrt(nc, in_maps, reps=3):
    """Multi-core PJRT run (mirrors bass2jax.run_bass_via_pjrt) with warm
    timing: inputs pre-staged on device, best-of-reps wall ns recorded."""
    import time
    import jax
    from jax.sharding import Mesh, PartitionSpec, NamedSharding
    from jax.experimental.shard_map import shard_map
    from concourse import bass2jax as b2j

    b2j.install_neuronx_cc_hook()
    partition_name = nc.partition_id_tensor.name if nc.partition_id_tensor else None
    in_names, out_names, out_avals, zero_outs = [], [], [], []
    for alloc in nc.m.functions[0].allocations:
        if not isinstance(alloc, mybir.MemoryLocationSet):
            continue
        name = alloc.memorylocations[0].name
        if alloc.kind == "ExternalInput":
            if name != partition_name:
                in_names.append(name)
        elif alloc.kind == "ExternalOutput":
            shape = tuple(alloc.tensor_shape)
            dtype = mybir.dt.np(alloc.dtype)
            out_avals.append(jax.core.ShapedArray(shape, dtype))
            zero_outs.append(np.zeros(shape, dtype))
            out_names.append(name)
    n_params = len(in_names)
    n_outs = len(out_avals)
    in_names.extend(out_names)
    if partition_name is not None:
        in_names.append(partition_name)
    donate = tuple(range(n_params, n_params + n_outs))

    def _body(*args):
        operands = list(args)
        if partition_name is not None:
            operands.append(b2j.partition_id_tensor())
        outs = b2j._bass_exec_p.bind(
            *operands, out_avals=tuple(out_avals), in_names=tuple(in_names),
            out_names=tuple(out_names), lowering_input_output_aliases=(),
            sim_require_finite=True, sim_require_nnan=True, nc=nc)
        return tuple(outs)

    devices = jax.devices()[:NCORES]
    mesh = Mesh(np.asarray(devices), ("core",))
    spec = NamedSharding(mesh, PartitionSpec("core"))
    sharded = jax.jit(
        shard_map(_body, mesh=mesh, in_specs=(PartitionSpec("core"),) * (n_params + n_outs),
                  out_specs=(PartitionSpec("core"),) * n_outs, check_rep=False),
        donate_argnums=donate, keep_unused=True)
    concat_in = [
        jax.device_put(
            np.concatenate([np.asarray(in_maps[c][nm]) for c in range(NCORES)], axis=0),
            spec)
        for nm in in_names[:n_params]]
    zsets = [
        [jax.device_put(np.zeros((NCORES * z.shape[0], *z.shape[1:]), z.dtype), spec)
         for z in zero_outs]
        for _ in range(reps + 1)]
    out_arrs = sharded(*concat_in, *zsets[0])   # warmup + keep results
    jax.block_until_ready(out_arrs)
    best = None
    for r in range(reps):
        t0 = time.perf_counter()
        o = sharded(*concat_in, *zsets[r + 1])
        jax.block_until_ready(o)
        dt = time.perf_counter() - t0
        best = dt if best is None else min(best, dt)
    last_exec_ns.append(int(best * 1e9))
    return [
        {nm: np.asarray(out_arrs[i]).reshape(NCORES, *out_avals[i].shape)[c]
         for i, nm in enumerate(out_names)}
        for c in range(NCORES)]


def _run(pd, in_maps):
    if TRACE:
        return _timed_pjrt(pd.nc, in_maps)
    res = run_bass_kernel_spmd(pd.nc, in_maps, list(range(NCORES)), trace=False)
    if res.exec_time_ns is not None:
        last_exec_ns.append(res.exec_time_ns)
    return res.results


def kernel(x_s, x_v, edge_attr_s, edge_attr_v, edge_index, params):
    last_exec_ns.clear()
    x_s = _f32(x_s)
    x_v = _f32(x_v)
    edge_attr_s = _f32(edge_attr_s)
    edge_attr_v = _f32(edge_attr_v)
    edge_index = np.asarray(edge_index)
    row = np.asarray(edge_index[0], dtype=np.int64)
    col = np.asarray(edge_index[1], dtype=np.int64)

    Wn0 = prep_simple(params['node0']); Wn1 = prep_simple(params['node1'])
    We0 = prep_simple(params['edge0']); We1 = prep_simple(params['edge1'])
    Wm = prep_fused(params['msg']); Wc = prep_simple(params['centroid'])
    Wo0 = prep_fused(params['out0']); Wo1 = prep_simple(params['out1'])
    ln_w = _f32(params['ln_w']); ln_b = _f32(params['ln_b'])

    # ---- host shard: bucket edges by destination-window of their row ----
    shards = []
    maxcnt = 0
    for k in range(NCORES):
        m = (row >= k * NPC) & (row < (k + 1) * NPC)
        idx = np.nonzero(m)[0]
        rl = row[idx] - k * NPC
        win = rl // 128
        order = np.argsort(win, kind='stable')
        idx = idx[order]
        swin = win[order]
        starts = np.searchsorted(swin, np.arange(NWIN_E))
        rank = np.arange(len(idx)) - starts[swin]
        cnt = np.bincount(swin, minlength=NWIN_E)
        maxcnt = max(maxcnt, int(cnt.max()))
        shards.append((idx, swin, rank, rl[order]))
    C = -(-maxcnt // 128)
    C = 4 * (-(-C // 4))
    G = NWIN_E * C
    Ec = G * 128

    eav_flat = edge_attr_v.reshape(E, 48)
    xs_fms, xv_fms, edge_ins = [], [], []
    for k in range(NCORES):
        idx, swin, rank, rl = shards[k]
        dest = swin * C * 128 + rank
        eas_fm = np.zeros((64, Ec), np.float32)
        eav_fm = np.zeros((48, Ec), np.float32)
        colp = np.zeros(Ec, np.int32)
        rrp = np.full(Ec, -1.0, np.float32)
        eas_fm[:, dest] = edge_attr_s[idx].T
        eav_fm[:, dest] = eav_flat[idx].T
        colp[dest] = col[idx]
        rrp[dest] = (rl - swin * 128).astype(np.float32)
        colv = np.ascontiguousarray(colp.reshape(G, 128).T)
        rrv = np.ascontiguousarray(rrp.reshape(G, 128).T)
        xs_fm = np.zeros((128, NPAD), np.float32)
        xv_fm = np.zeros((48, NPAD), np.float32)
        xs_fm[:, :NPC] = x_s[k * NPC:(k + 1) * NPC].T
        xv_fm[:, :NPC] = x_v.reshape(N, 48)[k * NPC:(k + 1) * NPC].T
        xs_fms.append(xs_fm)
        xv_fms.append(xv_fm)
        edge_ins.append(dict(eas=eas_fm, eav=eav_fm, colv=colv, rrv=rrv))

    # ---- launch 1: node net2 -> table slabs ----
    pa = build_phase_a(Wn0, Wn1)
    in1 = [dict(pa.common, xs=xs_fms[k], xv=xv_fms[k]) for k in range(NCORES)]
    res1 = _run(pa, in1)
    tbl_full = np.ascontiguousarray(
        np.concatenate([res1[k]['tbl'][:NPC] for k in range(NCORES)], axis=0))
    if DBG:
        dbg['C'] = C
        dbg['tbl'] = tbl_full

    # ---- launch 2: edges + scatter + node out ----
    pm = build_main(C, We0, We1, Wm, Wc, Wo0, Wo1, ln_w, ln_b)
    in2 = [dict(pm.common, tbl=tbl_full, xs=xs_fms[k], xv=xv_fms[k], **edge_ins[k])
           for k in range(NCORES)]
    res2 = _run(pm, in2)
    if DBG:
        dbg['acc'] = [res2[k]['dbgacc'] for k in range(NCORES)]
        for nm in ("hes", "hev", "rs", "rv", "ms", "mv"):
            dbg[nm] = [res2[k][f"dbg_{nm}"] for k in range(NCORES)]
    out = np.concatenate([res2[k]['out'][:NPC] for k in range(NCORES)], axis=0)
    os_ = np.ascontiguousarray(out[:, :128])
    ov = np.ascontiguousarray(out[:, 128:]).reshape(N, 16, 3)
    return os_, ov
